# revision 4
# baseline (speedup 1.0000x reference)
"""GCN-with-edge-features kernel for 8 Trainium2 cores.

Strategy (per sharding hint): data-parallel over edges. The dominant
compute is the two edge-net MLPs:
  theta1 = relu(ea @ W1a + b1a) @ W1b + b1b   [100000, 1024]
  theta2 = relu(ea @ W2a + b2a) @ W2b + b2b   [100000, 2048]
(~630 GFLOP total). These run on-device, 12500 edges per core
(padded to 12800 = 25 chunks of 512). The remaining ops (gather,
per-edge matvec, segment-mean scatter, graph pooling, small MLP)
are bandwidth-trivial and run on host.
"""
import numpy as np

import sys
for p in ("/opt/trn_rl_repo",):
    if p not in sys.path:
        sys.path.append(p)

from concourse import bass, bacc, mybir, tile
from concourse import bass_utils

E = 100000
N = 50000
NG = 2000
F_IN = 32
EF = 16
H = 32
H2 = 64
NC = 8
EPC = E // NC          # 12500 edges per core
CH = 512
NCHUNK = 25
EP = CH * NCHUNK       # 12800 padded edges per core
D1 = H * F_IN          # 1024
D2 = H * H2            # 2048

_F32 = mybir.dt.float32
_RELU = mybir.ActivationFunctionType.Relu
_IDENT = mybir.ActivationFunctionType.Identity

_NC_CACHE = {}


def _build_nc():
    nc = bacc.Bacc(None, target_bir_lowering=False)

    eaT_d = nc.dram_tensor("eaT", [EF, EP], _F32, kind="ExternalInput")
    W1a_d = nc.dram_tensor("W1a", [EF, D1], _F32, kind="ExternalInput")
    W2a_d = nc.dram_tensor("W2a", [EF, D1], _F32, kind="ExternalInput")
    W1b_d = nc.dram_tensor("W1b", [D1, D1], _F32, kind="ExternalInput")
    W2b_d = nc.dram_tensor("W2b", [D1, D2], _F32, kind="ExternalInput")
    b1a_d = nc.dram_tensor("b1a", [128, 8], _F32, kind="ExternalInput")
    b2a_d = nc.dram_tensor("b2a", [128, 8], _F32, kind="ExternalInput")
    b1b_d = nc.dram_tensor("b1b", [128, 8], _F32, kind="ExternalInput")
    b2b_d = nc.dram_tensor("b2b", [128, 16], _F32, kind="ExternalInput")
    t1T_d = nc.dram_tensor("t1T", [D1, EP], _F32, kind="ExternalOutput")
    t2T_d = nc.dram_tensor("t2T", [D2, EP], _F32, kind="ExternalOutput")

    with tile.TileContext(nc) as tc:
        with (
            tc.tile_pool(name="w", bufs=1) as wpool,
            tc.tile_pool(name="ea", bufs=3) as eapool,
            tc.tile_pool(name="h", bufs=2) as hpool,
            tc.tile_pool(name="o", bufs=4) as opool,
            tc.tile_pool(name="ps", bufs=4, space=bass.MemorySpace.PSUM) as pspool,
        ):
            w1a = wpool.tile([EF, D1], _F32)
            nc.sync.dma_start(w1a[:], W1a_d[:])
            w2a = wpool.tile([EF, D1], _F32)
            nc.sync.dma_start(w2a[:], W2a_d[:])
            # W1b as [128, 8*1024]: k-th contraction slice at cols k*1024..
            w1b = wpool.tile([128, 8 * D1], _F32)
            for k in range(8):
                nc.sync.dma_start(
                    w1b[:, k * D1:(k + 1) * D1], W1b_d[k * 128:(k + 1) * 128, :]
                )
            w2b = wpool.tile([128, 8 * D2], _F32)
            for k in range(8):
                nc.sync.dma_start(
                    w2b[:, k * D2:(k + 1) * D2], W2b_d[k * 128:(k + 1) * 128, :]
                )
            b1a = wpool.tile([128, 8], _F32)
            nc.sync.dma_start(b1a[:], b1a_d[:])
            b2a = wpool.tile([128, 8], _F32)
            nc.sync.dma_start(b2a[:], b2a_d[:])
            b1b = wpool.tile([128, 8], _F32)
            nc.sync.dma_start(b1b[:], b1b_d[:])
            b2b = wpool.tile([128, 16], _F32)
            nc.sync.dma_start(b2b[:], b2b_d[:])

            for c in range(NCHUNK):
                s = c * CH
                ea = eapool.tile([EF, CH], _F32)
                nc.sync.dma_start(ea[:], eaT_d[:, s:s + CH])

                h1 = hpool.tile([128, 8 * CH], _F32)
                h2 = hpool.tile([128, 8 * CH], _F32)
                for j in range(8):
                    ps = pspool.tile([128, CH], _F32)
                    nc.tensor.matmul(
                        ps[:], w1a[:, j * 128:(j + 1) * 128], ea[:],
                        start=True, stop=True,
                    )
                    nc.scalar.activation(
                        h1[:, j * CH:(j + 1) * CH], ps[:], _RELU,
                        bias=b1a[:, j:j + 1],
                    )
                    ps2 = pspool.tile([128, CH], _F32)
                    nc.tensor.matmul(
                        ps2[:], w2a[:, j * 128:(j + 1) * 128], ea[:],
                        start=True, stop=True,
                    )
                    nc.scalar.activation(
                        h2[:, j * CH:(j + 1) * CH], ps2[:], _RELU,
                        bias=b2a[:, j:j + 1],
                    )

                for j in range(8):
                    ps = pspool.tile([128, CH], _F32)
                    for k in range(8):
                        nc.tensor.matmul(
                            ps[:],
                            w1b[:, k * D1 + j * 128:k * D1 + (j + 1) * 128],
                            h1[:, k * CH:(k + 1) * CH],
                            start=(k == 0), stop=(k == 7),
                        )
                    o = opool.tile([128, CH], _F32)
                    nc.scalar.activation(o[:], ps[:], _IDENT, bias=b1b[:, j:j + 1])
                    nc.sync.dma_start(t1T_d[j * 128:(j + 1) * 128, s:s + CH], o[:])

                for j in range(16):
                    ps = pspool.tile([128, CH], _F32)
                    for k in range(8):
                        nc.tensor.matmul(
                            ps[:],
                            w2b[:, k * D2 + j * 128:k * D2 + (j + 1) * 128],
                            h2[:, k * CH:(k + 1) * CH],
                            start=(k == 0), stop=(k == 7),
                        )
                    o = opool.tile([128, CH], _F32)
                    nc.scalar.activation(o[:], ps[:], _IDENT, bias=b2b[:, j:j + 1])
                    nc.sync.dma_start(t2T_d[j * 128:(j + 1) * 128, s:s + CH], o[:])

    nc.compile()
    return nc


def _get_nc():
    if "nc" not in _NC_CACHE:
        _NC_CACHE["nc"] = _build_nc()
    return _NC_CACHE["nc"]


def compiled_ncs():
    return [_get_nc()]


def _relu(v):
    return np.maximum(v, 0.0)


def _segmean(vals, idx, n):
    s = np.zeros((n, vals.shape[1]), np.float32)
    np.add.at(s, idx, vals)
    c = np.bincount(idx, minlength=n).astype(np.float32)
    return s / np.maximum(c, 1.0)[:, None]


def kernel(**inputs):
    x = np.asarray(inputs["x"], np.float32)
    edge_index = np.asarray(inputs["edge_index"])
    eap = np.asarray(inputs["edge_attr_packed"])
    batch = np.asarray(inputs["batch"])
    W1a = np.ascontiguousarray(inputs["W1a"], np.float32)
    W1b = np.ascontiguousarray(inputs["W1b"], np.float32)
    W2a = np.ascontiguousarray(inputs["W2a"], np.float32)
    W2b = np.ascontiguousarray(inputs["W2b"], np.float32)
    b1a = np.asarray(inputs["b1a"], np.float32)
    b1b = np.asarray(inputs["b1b"], np.float32)
    b2a = np.asarray(inputs["b2a"], np.float32)
    b2b = np.asarray(inputs["b2b"], np.float32)
    root1 = np.asarray(inputs["root1"], np.float32)
    bias1 = np.asarray(inputs["bias1"], np.float32)
    root2 = np.asarray(inputs["root2"], np.float32)
    bias2 = np.asarray(inputs["bias2"], np.float32)

    # MSB-first bit unpack -> [E, 16]
    shifts = np.arange(7, -1, -1, dtype=np.int32)
    ea = ((eap[:, :, None].astype(np.int32) >> shifts) & 1).reshape(E, -1)
    ea = ea.astype(np.float32)

    def bt(b, cols):
        return np.ascontiguousarray(b.reshape(cols, 128).T)

    nc = _get_nc()
    in_maps = []
    for i in range(NC):
        sh = np.zeros((EP, EF), np.float32)
        sh[:EPC] = ea[i * EPC:(i + 1) * EPC]
        in_maps.append(dict(
            eaT=np.ascontiguousarray(sh.T),
            W1a=W1a, W2a=W2a, W1b=W1b, W2b=W2b,
            b1a=bt(b1a, 8), b2a=bt(b2a, 8),
            b1b=bt(b1b, 8), b2b=bt(b2b, 16),
        ))
    res = bass_utils.run_bass_kernel_spmd(nc, in_maps, core_ids=list(range(NC)))
    results = res.results
    theta1 = np.concatenate(
        [results[i]["t1T"][:, :EPC].T for i in range(NC)], axis=0)
    theta2 = np.concatenate(
        [results[i]["t2T"][:, :EPC].T for i in range(NC)], axis=0)

    src, dst = edge_index[0], edge_index[1]

    msg1 = np.einsum("ei,eio->eo", x[src], theta1.reshape(E, F_IN, H),
                     optimize=True).astype(np.float32)
    h = _relu(_segmean(msg1, dst, N) + x @ root1 + bias1)

    msg2 = np.einsum("ei,eio->eo", h[src], theta2.reshape(E, H, H2),
                     optimize=True).astype(np.float32)
    h = _relu(_segmean(msg2, dst, N) + h @ root2 + bias2)

    g = _segmean(h, batch, NG)
    g = _relu(g @ np.asarray(inputs["fcW1"], np.float32) + np.asarray(inputs["fcb1"], np.float32))
    g = _relu(g @ np.asarray(inputs["fcW2"], np.float32) + np.asarray(inputs["fcb2"], np.float32))
    g = _relu(g @ np.asarray(inputs["fcW3"], np.float32) + np.asarray(inputs["fcb3"], np.float32))
    return (g @ np.asarray(inputs["fcW4"], np.float32) + np.asarray(inputs["fcb4"], np.float32)).astype(np.float32)



# revision 11
# speedup vs baseline: 3.4599x; 3.4599x over previous
"""GCN-with-edge-features kernel for 8 Trainium2 cores.

Data-parallel over edges (12500/core, padded to 12800). Two device
launches:
  A: h1 = relu(ea@W1a+b1a); theta1 = h1@W1b+b1b (kept in PSUM);
     msg1[e,o] = sum_i x[src[e],i] * theta1[e,i,o]   -> [E, 32]
  B: same with W2a/W2b and h[src]                    -> [E, 64]
Matmuls run as float32r (1 cycle/row). theta never leaves the chip;
the per-edge contraction is a DVE strided multiply + reduce with edges
on partitions. Host does only the segment-mean scatters, the tiny
root/bias adds, and the final 2000-row MLP.
"""
import numpy as np

import sys
for p in ("/opt/trn_rl_repo",):
    if p not in sys.path:
        sys.path.append(p)

from concourse import bass, bacc, mybir, tile
from concourse import bass_utils

E = 100000
N = 50000
NG = 2000
F_IN = 32
EF = 16
H = 32
H2 = 64
NC = 8
EPC = E // NC          # 12500 edges per core
CH = 512
NCHUNK = 25
EP = CH * NCHUNK       # 12800 padded edges per core
EB = CH // 128         # 4 edge blocks per chunk
D1 = H * F_IN          # 1024
D2 = H * H2            # 2048

_F32 = mybir.dt.float32
_F32R = mybir.dt.float32r
_RELU = mybir.ActivationFunctionType.Relu
_AX_X = mybir.AxisListType.X
_MUL = mybir.AluOpType.mult
_ADD = mybir.AluOpType.add

_NC_CACHE = {}


def _build_layer(tag, fin, fout):
    """One NNConv layer program: edge-net MLP + per-edge contraction.

    fin: per-edge input feature dim (32), fout: output dim (32 or 64).
    Edge-net: hmid = relu(ea @ Wa + ba)  [D1, e]
              theta = hmid @ Wb + bb     [e, fin*fout] (PSUM only)
              msg[e, o] = sum_i xs[e, i] * theta[e, i*fout + o]
    Wb/bb are expected with columns reordered to (o, i): col o*fin + i.
    Each 512-col block (16 outputs x 32 contraction) is one PSUM bank.
    """
    DW = fin * fout                 # 1024 or 2048
    NQ = DW // 512                  # 2 for layer 1, 4 for layer 2

    nc = bacc.Bacc(None, target_bir_lowering=False)

    eaT_d = nc.dram_tensor("eaT", [EF, EP], _F32R, kind="ExternalInput")
    xsE_d = nc.dram_tensor("xsE", [EP, fin], _F32, kind="ExternalInput")
    Wa_d = nc.dram_tensor("Wa", [EF, D1], _F32R, kind="ExternalInput")
    ba_d = nc.dram_tensor("ba", [128, 8], _F32, kind="ExternalInput")
    Wb_d = nc.dram_tensor("Wb", [D1, DW], _F32R, kind="ExternalInput")
    bb_d = nc.dram_tensor("bb", [1, DW], _F32R, kind="ExternalInput")
    ones_d = nc.dram_tensor("ones", [1, 128], _F32R, kind="ExternalInput")
    msg_d = nc.dram_tensor("msg", [EP, fout], _F32, kind="ExternalOutput")

    with tile.TileContext(nc) as tc:
        with (
            tc.tile_pool(name="w", bufs=1) as wpool,
            tc.tile_pool(name="ea", bufs=3) as eapool,
            tc.tile_pool(name="h", bufs=2) as hpool,
            tc.tile_pool(name="xs", bufs=4) as xspool,
            tc.tile_pool(name="pr", bufs=3) as prpool,
            tc.tile_pool(name="o", bufs=4) as opool,
            tc.tile_pool(name="psh", bufs=2, space=bass.MemorySpace.PSUM) as pshpool,
            tc.tile_pool(name="pst", bufs=3, space=bass.MemorySpace.PSUM) as pstpool,
        ):
            wa = wpool.tile([EF, D1], _F32R)
            nc.sync.dma_start(wa[:], Wa_d[:])
            wb = wpool.tile([128, 8 * DW], _F32R)
            for k in range(8):
                nc.sync.dma_start(
                    wb[:, k * DW:(k + 1) * DW], Wb_d[k * 128:(k + 1) * 128, :]
                )
            ba = wpool.tile([128, 8], _F32)
            nc.sync.dma_start(ba[:], ba_d[:])
            bb = wpool.tile([1, DW], _F32R)
            nc.sync.dma_start(bb[:], bb_d[:])
            ones = wpool.tile([1, 128], _F32R)
            nc.sync.dma_start(ones[:], ones_d[:])

            for c in range(NCHUNK):
                s = c * CH
                ea = eapool.tile([EF, CH], _F32R)
                nc.sync.dma_start(ea[:], eaT_d[:, s:s + CH])

                # hmid = relu(Wa.T @ ea + ba): [1024, CH] as 8 tiles
                hm = hpool.tile([128, 8 * CH], _F32R)
                for j in range(8):
                    ps = pshpool.tile([128, CH], _F32)
                    nc.tensor.matmul(
                        ps[:],
                        wa[:, j * 128:(j + 1) * 128],
                        ea[:],
                        start=True, stop=True,
                    )
                    nc.scalar.activation(
                        hm[:, j * CH:(j + 1) * CH], ps[:], _RELU,
                        bias=ba[:, j:j + 1],
                    )

                for eb in range(EB):
                    xs = xspool.tile([128, fin], _F32)
                    nc.sync.dma_start(xs[:], xsE_d[s + eb * 128:s + (eb + 1) * 128, :])
                    msg = opool.tile([128, fout], _F32)

                    for qb in range(NQ):
                        # theta block (sans bias): [128 edges, 512] =
                        # hm.T @ Wb_blk, outputs o in [16qb, 16qb+16), all i.
                        # The edge-net bias term sum_i xs[e,i]*bb[i,o] is a
                        # tiny host GEMM added on the host side.
                        P = pstpool.tile([128, 512], _F32)
                        for k in range(8):
                            nc.tensor.matmul(
                                P[:],
                                hm[:, k * CH + eb * 128:k * CH + (eb + 1) * 128],
                                wb[:, k * DW + qb * 512:k * DW + (qb + 1) * 512],
                                start=(k == 0), stop=(k == 7),
                            )
                        # msg[:, o] = sum_i theta[:, o*fin + i] * xs[:, i]
                        prod = prpool.tile([128, 512], _F32)
                        nc.vector.tensor_tensor(
                            prod[:].rearrange("p (o i) -> p o i", o=16, i=32),
                            P[:].rearrange("p (o i) -> p o i", o=16, i=32),
                            xs[:].unsqueeze(1).broadcast_to([128, 16, fin]),
                            _MUL,
                        )
                        nc.vector.reduce_sum(
                            msg[:, qb * 16:(qb + 1) * 16],
                            prod[:].rearrange("p (o i) -> p o i", o=16, i=32),
                            _AX_X,
                        )
                    nc.sync.dma_start(
                        msg_d[s + eb * 128:s + (eb + 1) * 128, :], msg[:]
                    )

    nc.compile()
    return nc


def _get_nc(tag):
    if tag not in _NC_CACHE:
        fin, fout = (F_IN, H) if tag == "A" else (H, H2)
        _NC_CACHE[tag] = _build_layer(tag, fin, fout)
    return _NC_CACHE[tag]


def compiled_ncs():
    return [_get_nc("A"), _get_nc("B")]


def _relu(v):
    return np.maximum(v, 0.0)


def _segmean(vals, idx, n):
    s = np.zeros((n, vals.shape[1]), np.float32)
    np.add.at(s, idx, vals)
    c = np.bincount(idx, minlength=n).astype(np.float32)
    return s / np.maximum(c, 1.0)[:, None]


def _reorder_oi(Wb, bb, fin, fout):
    """Reorder edge-net output cols from (i, o) = i*fout + o to (o, i) =
    o*fin + i, so each 512-col block is 16 complete outputs."""
    W = Wb.reshape(D1, fin, fout).transpose(0, 2, 1).reshape(D1, fin * fout)
    b = bb.reshape(fin, fout).T.reshape(-1)
    return np.ascontiguousarray(W), np.ascontiguousarray(b)[None, :]


def _run_layer(tag, ea, feat_src, Wa, ba, Wb_r, bb_r, fout):
    """ea: [E,16] f32; feat_src: [E, fin] f32 (features gathered at src)."""
    nc = _get_nc(tag)
    baT = np.ascontiguousarray(ba.reshape(8, 128).T)
    in_maps = []
    for i in range(NC):
        eaT = np.zeros((EF, EP), np.float32)
        eaT[:, :EPC] = ea[i * EPC:(i + 1) * EPC].T
        xsE = np.zeros((EP, feat_src.shape[1]), np.float32)
        xsE[:EPC] = feat_src[i * EPC:(i + 1) * EPC]
        in_maps.append(dict(eaT=eaT, xsE=xsE, Wa=Wa, ba=baT, Wb=Wb_r, bb=bb_r,
                            ones=np.ones((1, 128), np.float32)))
    res = bass_utils.run_bass_kernel_spmd(nc, in_maps, core_ids=list(range(NC)))
    return np.concatenate([res.results[i]["msg"][:EPC] for i in range(NC)], axis=0)


def kernel(**inputs):
    x = np.asarray(inputs["x"], np.float32)
    edge_index = np.asarray(inputs["edge_index"])
    eap = np.asarray(inputs["edge_attr_packed"])
    batch = np.asarray(inputs["batch"])
    W1a = np.ascontiguousarray(inputs["W1a"], np.float32)
    W1b = np.ascontiguousarray(inputs["W1b"], np.float32)
    W2a = np.ascontiguousarray(inputs["W2a"], np.float32)
    W2b = np.ascontiguousarray(inputs["W2b"], np.float32)
    b1a = np.asarray(inputs["b1a"], np.float32)
    b1b = np.asarray(inputs["b1b"], np.float32)
    b2a = np.asarray(inputs["b2a"], np.float32)
    b2b = np.asarray(inputs["b2b"], np.float32)
    root1 = np.asarray(inputs["root1"], np.float32)
    bias1 = np.asarray(inputs["bias1"], np.float32)
    root2 = np.asarray(inputs["root2"], np.float32)
    bias2 = np.asarray(inputs["bias2"], np.float32)

    # MSB-first bit unpack -> [E, 16]
    shifts = np.arange(7, -1, -1, dtype=np.int32)
    ea = ((eap[:, :, None].astype(np.int32) >> shifts) & 1).reshape(E, -1)
    ea = ea.astype(np.float32)

    src, dst = edge_index[0], edge_index[1]

    W1b_r, b1b_r = _reorder_oi(W1b, b1b, F_IN, H)
    msg1 = _run_layer("A", ea, x[src], W1a, b1a, W1b_r, b1b_r, H)
    h = _relu(_segmean(msg1, dst, N) + x @ root1 + bias1)

    W2b_r, b2b_r = _reorder_oi(W2b, b2b, H, H2)
    msg2 = _run_layer("B", ea, h[src], W2a, b2a, W2b_r, b2b_r, H2)
    h = _relu(_segmean(msg2, dst, N) + h @ root2 + bias2)

    g = _segmean(h, batch, NG)
    g = _relu(g @ np.asarray(inputs["fcW1"], np.float32) + np.asarray(inputs["fcb1"], np.float32))
    g = _relu(g @ np.asarray(inputs["fcW2"], np.float32) + np.asarray(inputs["fcb2"], np.float32))
    g = _relu(g @ np.asarray(inputs["fcW3"], np.float32) + np.asarray(inputs["fcb3"], np.float32))
    return (g @ np.asarray(inputs["fcW4"], np.float32) + np.asarray(inputs["fcb4"], np.float32)).astype(np.float32)


# revision 14
# speedup vs baseline: 5.6832x; 1.6426x over previous
"""GCN-with-edge-features kernel for 8 Trainium2 cores.

Data-parallel over edges (12500/core, padded to 12800). Two device
launches:
  A: h1 = relu(ea@W1a+b1a); theta1 = h1@W1b+b1b (kept in PSUM);
     msg1[e,o] = sum_i x[src[e],i] * theta1[e,i,o]   -> [E, 32]
  B: same with W2a/W2b and h[src]                    -> [E, 64]
Matmuls run as float32r (1 cycle/row). theta never leaves the chip;
the per-edge contraction is a DVE strided multiply + reduce with edges
on partitions. Host does only the segment-mean scatters, the tiny
root/bias adds, and the final 2000-row MLP.
"""
import numpy as np

import sys
for p in ("/opt/trn_rl_repo",):
    if p not in sys.path:
        sys.path.append(p)

from concourse import bass, bacc, mybir, tile
from concourse import bass_utils

E = 100000
N = 50000
NG = 2000
F_IN = 32
EF = 16
H = 32
H2 = 64
NC = 8
EPC = E // NC          # 12500 edges per core
CH = 512
NCHUNK = 25
EP = CH * NCHUNK       # 12800 padded edges per core
EB = CH // 128         # 4 edge blocks per chunk
D1 = H * F_IN          # 1024
D2 = H * H2            # 2048

_F32 = mybir.dt.float32
_F32R = mybir.dt.float32r
_F8 = mybir.dt.float8e4
_DR = mybir.MatmulPerfMode.DoubleRow

USE_FP8 = True          # fp8e4m3 + DoubleRow for the big GEMMs
FP8_WSCALE = 64.0       # weight pre-scale (W ~ +-1/32 underflows e4m3)
MUL_ENGINE = "vector"   # engine for the per-edge multiply
_RELU = mybir.ActivationFunctionType.Relu
_AX_X = mybir.AxisListType.X
_MUL = mybir.AluOpType.mult
_ADD = mybir.AluOpType.add

_NC_CACHE = {}


def _build_layer(tag, fin, fout):
    """One NNConv layer program: edge-net MLP + per-edge contraction.

    fin: per-edge input feature dim (32), fout: output dim (32 or 64).
    Edge-net: hmid = relu(ea @ Wa + ba)  [D1, e]
              theta = hmid @ Wb + bb     [e, fin*fout] (PSUM only)
              msg[e, o] = sum_i xs[e, i] * theta[e, i*fout + o]
    Wb/bb are expected with columns reordered to (o, i): col o*fin + i.
    Each 512-col block (16 outputs x 32 contraction) is one PSUM bank.
    """
    DW = fin * fout                 # 1024 or 2048
    NQ = DW // 512                  # 2 for layer 1, 4 for layer 2
    wdt = _F8 if USE_FP8 else _F32R

    nc = bacc.Bacc(None, target_bir_lowering=False)

    eaT_d = nc.dram_tensor("eaT", [EF, EP], _F32R, kind="ExternalInput")
    xsE_d = nc.dram_tensor("xsE", [EP, fin], _F32, kind="ExternalInput")
    Wa_d = nc.dram_tensor("Wa", [EF, D1], _F32R, kind="ExternalInput")
    ba_d = nc.dram_tensor("ba", [128, 8], _F32, kind="ExternalInput")
    Wb_d = nc.dram_tensor("Wb", [D1, DW], wdt, kind="ExternalInput")
    msg_d = nc.dram_tensor("msg", [EP, fout], _F32, kind="ExternalOutput")

    with tile.TileContext(nc) as tc:
        with (
            tc.tile_pool(name="w", bufs=1) as wpool,
            tc.tile_pool(name="ea", bufs=3) as eapool,
            tc.tile_pool(name="h", bufs=2) as hpool,
            tc.tile_pool(name="xs", bufs=4) as xspool,
            tc.tile_pool(name="pr", bufs=3) as prpool,
            tc.tile_pool(name="o", bufs=4) as opool,
            tc.tile_pool(name="psh", bufs=2, space=bass.MemorySpace.PSUM) as pshpool,
            tc.tile_pool(name="pst", bufs=3, space=bass.MemorySpace.PSUM) as pstpool,
        ):
            wa = wpool.tile([EF, D1], _F32R)
            nc.sync.dma_start(wa[:], Wa_d[:])
            wb = wpool.tile([128, 8 * DW], wdt)
            for k in range(8):
                nc.sync.dma_start(
                    wb[:, k * DW:(k + 1) * DW], Wb_d[k * 128:(k + 1) * 128, :]
                )
            ba = wpool.tile([128, 8], _F32)
            nc.sync.dma_start(ba[:], ba_d[:])

            for c in range(NCHUNK):
                s = c * CH
                ea = eapool.tile([EF, CH], _F32R)
                nc.sync.dma_start(ea[:], eaT_d[:, s:s + CH])

                # hmid = relu(Wa.T @ ea + ba): [1024, CH] as 8 tiles
                hm = hpool.tile([128, 8 * CH], wdt)
                for j in range(8):
                    ps = pshpool.tile([128, CH], _F32)
                    nc.tensor.matmul(
                        ps[:],
                        wa[:, j * 128:(j + 1) * 128],
                        ea[:],
                        start=True, stop=True,
                    )
                    nc.scalar.activation(
                        hm[:, j * CH:(j + 1) * CH], ps[:], _RELU,
                        bias=ba[:, j:j + 1],
                    )

                for eb in range(EB):
                    xs = xspool.tile([128, fin], _F32)
                    nc.sync.dma_start(xs[:], xsE_d[s + eb * 128:s + (eb + 1) * 128, :])
                    msg = opool.tile([128, fout], _F32)

                    for qb in range(NQ):
                        # theta block (sans bias): [128 edges, 512] =
                        # hm.T @ Wb_blk, outputs o in [16qb, 16qb+16), all i.
                        # The edge-net bias term sum_i xs[e,i]*bb[i,o] is a
                        # tiny host GEMM added on the host side.
                        P = pstpool.tile([128, 512], _F32)
                        if USE_FP8:
                            # DoubleRow: 2 contraction rows per partition;
                            # sub-row s of group g = hm tile 2g+s / Wb row
                            # block 2g+s (pairing is per-sub, order free).
                            hm3 = hm[:].rearrange("p (t e) -> p t e", t=8)
                            wb3 = wb[:].rearrange("p (t d) -> p t d", t=8)
                            for g in range(4):
                                nc.tensor.matmul(
                                    P[:],
                                    hm3[:, 2 * g:2 * g + 2, eb * 128:(eb + 1) * 128],
                                    wb3[:, 2 * g:2 * g + 2, qb * 512:(qb + 1) * 512],
                                    start=(g == 0), stop=(g == 3),
                                    perf_mode=_DR,
                                )
                        else:
                            for k in range(8):
                                nc.tensor.matmul(
                                    P[:],
                                    hm[:, k * CH + eb * 128:k * CH + (eb + 1) * 128],
                                    wb[:, k * DW + qb * 512:k * DW + (qb + 1) * 512],
                                    start=(k == 0), stop=(k == 7),
                                )
                        # msg[:, o] = sum_i theta[:, o*fin + i] * xs[:, i]
                        prod = prpool.tile([128, 512], _F32)
                        mul_eng = nc.gpsimd if MUL_ENGINE == "gpsimd" else nc.vector
                        mul_eng.tensor_tensor(
                            prod[:].rearrange("p (o i) -> p o i", o=16, i=32),
                            P[:].rearrange("p (o i) -> p o i", o=16, i=32),
                            xs[:].unsqueeze(1).broadcast_to([128, 16, fin]),
                            _MUL,
                        )
                        nc.vector.reduce_sum(
                            msg[:, qb * 16:(qb + 1) * 16],
                            prod[:].rearrange("p (o i) -> p o i", o=16, i=32),
                            _AX_X,
                        )
                    nc.sync.dma_start(
                        msg_d[s + eb * 128:s + (eb + 1) * 128, :], msg[:]
                    )

    nc.compile()
    return nc


def _get_nc(tag):
    if tag not in _NC_CACHE:
        fin, fout = (F_IN, H) if tag == "A" else (H, H2)
        _NC_CACHE[tag] = _build_layer(tag, fin, fout)
    return _NC_CACHE[tag]


def compiled_ncs():
    return [_get_nc("A"), _get_nc("B")]


def _relu(v):
    return np.maximum(v, 0.0)


def _segmean(vals, idx, n):
    s = np.zeros((n, vals.shape[1]), np.float32)
    np.add.at(s, idx, vals)
    c = np.bincount(idx, minlength=n).astype(np.float32)
    return s / np.maximum(c, 1.0)[:, None]


def _reorder_oi(Wb, fin, fout):
    """Reorder edge-net output cols from (i, o) = i*fout + o to (o, i) =
    o*fin + i, so each 512-col block is 16 complete outputs."""
    W = Wb.reshape(D1, fin, fout).transpose(0, 2, 1).reshape(D1, fin * fout)
    return np.ascontiguousarray(W)


def _run_layer(tag, ea, feat_src, Wa, ba, Wb_r, bb, fout):
    """ea: [E,16] f32; feat_src: [E, fin] f32 (features gathered at src).
    bb: raw edge-net output bias [fin*fout] - applied host-side as
    msg += feat_src @ bb.reshape(fin, fout)."""
    nc = _get_nc(tag)
    baT = np.ascontiguousarray(ba.reshape(8, 128).T)
    if USE_FP8:
        import ml_dtypes
        Wb_fin = (Wb_r * FP8_WSCALE).astype(ml_dtypes.float8_e4m3)
    else:
        Wb_fin = Wb_r
    in_maps = []
    for i in range(NC):
        eaT = np.zeros((EF, EP), np.float32)
        eaT[:, :EPC] = ea[i * EPC:(i + 1) * EPC].T
        xsE = np.zeros((EP, feat_src.shape[1]), np.float32)
        xsE[:EPC] = feat_src[i * EPC:(i + 1) * EPC]
        if USE_FP8:
            xsE /= FP8_WSCALE
        in_maps.append(dict(eaT=eaT, xsE=xsE, Wa=Wa, ba=baT, Wb=Wb_fin))
    res = bass_utils.run_bass_kernel_spmd(nc, in_maps, core_ids=list(range(NC)))
    msg = np.concatenate([res.results[i]["msg"][:EPC] for i in range(NC)], axis=0)
    fin = feat_src.shape[1]
    return msg + feat_src @ bb.reshape(fin, fout)


def kernel(**inputs):
    x = np.asarray(inputs["x"], np.float32)
    edge_index = np.asarray(inputs["edge_index"])
    eap = np.asarray(inputs["edge_attr_packed"])
    batch = np.asarray(inputs["batch"])
    W1a = np.ascontiguousarray(inputs["W1a"], np.float32)
    W1b = np.ascontiguousarray(inputs["W1b"], np.float32)
    W2a = np.ascontiguousarray(inputs["W2a"], np.float32)
    W2b = np.ascontiguousarray(inputs["W2b"], np.float32)
    b1a = np.asarray(inputs["b1a"], np.float32)
    b1b = np.asarray(inputs["b1b"], np.float32)
    b2a = np.asarray(inputs["b2a"], np.float32)
    b2b = np.asarray(inputs["b2b"], np.float32)
    root1 = np.asarray(inputs["root1"], np.float32)
    bias1 = np.asarray(inputs["bias1"], np.float32)
    root2 = np.asarray(inputs["root2"], np.float32)
    bias2 = np.asarray(inputs["bias2"], np.float32)

    # MSB-first bit unpack -> [E, 16]
    shifts = np.arange(7, -1, -1, dtype=np.int32)
    ea = ((eap[:, :, None].astype(np.int32) >> shifts) & 1).reshape(E, -1)
    ea = ea.astype(np.float32)

    src, dst = edge_index[0], edge_index[1]

    W1b_r = _reorder_oi(W1b, F_IN, H)
    msg1 = _run_layer("A", ea, x[src], W1a, b1a, W1b_r, b1b, H)
    h = _relu(_segmean(msg1, dst, N) + x @ root1 + bias1)

    W2b_r = _reorder_oi(W2b, H, H2)
    msg2 = _run_layer("B", ea, h[src], W2a, b2a, W2b_r, b2b, H2)
    h = _relu(_segmean(msg2, dst, N) + h @ root2 + bias2)

    g = _segmean(h, batch, NG)
    g = _relu(g @ np.asarray(inputs["fcW1"], np.float32) + np.asarray(inputs["fcb1"], np.float32))
    g = _relu(g @ np.asarray(inputs["fcW2"], np.float32) + np.asarray(inputs["fcb2"], np.float32))
    g = _relu(g @ np.asarray(inputs["fcW3"], np.float32) + np.asarray(inputs["fcb3"], np.float32))
    return (g @ np.asarray(inputs["fcW4"], np.float32) + np.asarray(inputs["fcb4"], np.float32)).astype(np.float32)


# revision 22
# speedup vs baseline: 8.0607x; 1.4183x over previous
"""GCN-with-edge-features kernel for 8 Trainium2 cores.

Data-parallel over edges (12500/core, padded to 12800). Two device
launches:
  A: h1 = relu(ea@W1a+b1a); theta1 = h1@W1b+b1b (kept in PSUM);
     msg1[e,o] = sum_i x[src[e],i] * theta1[e,i,o]   -> [E, 32]
  B: same with W2a/W2b and h[src]                    -> [E, 64]
Matmuls run as float32r (1 cycle/row). theta never leaves the chip;
the per-edge contraction is a DVE strided multiply + reduce with edges
on partitions. Host does only the segment-mean scatters, the tiny
root/bias adds, and the final 2000-row MLP.
"""
import numpy as np

import sys
for p in ("/opt/trn_rl_repo",):
    if p not in sys.path:
        sys.path.append(p)

from concourse import bass, bacc, mybir, tile
from concourse import bass_utils

E = 100000
N = 50000
NG = 2000
F_IN = 32
EF = 16
H = 32
H2 = 64
NC = 8
EPC = E // NC          # 12500 edges per core
CH = 512
NCHUNK = 25
EP = CH * NCHUNK       # 12800 padded edges per core
EB = CH // 128         # 4 edge blocks per chunk
D1 = H * F_IN          # 1024
D2 = H * H2            # 2048

_F32 = mybir.dt.float32
_F32R = mybir.dt.float32r
_F8 = mybir.dt.float8e4
_BF16 = mybir.dt.bfloat16
_DR = mybir.MatmulPerfMode.DoubleRow

USE_FP8 = True          # fp8e4m3 + DoubleRow for the big GEMMs
FP8_WSCALE = 64.0       # Wb pre-scale (W ~ +-1/32 underflows e4m3)
WA_SCALE = 16.0         # Wa pre-scale for the fp8 edge-net input GEMM
MUL_ENGINE = "vector"   # engine for the per-edge multiply
_RELU = mybir.ActivationFunctionType.Relu
_AX_X = mybir.AxisListType.X
_MUL = mybir.AluOpType.mult
_ADD = mybir.AluOpType.add

_NC_CACHE = {}


def _build_layer(tag, fin, fout):
    """One NNConv layer program: edge-net MLP + per-edge contraction.

    Orientation: theta^T [(o,i), e] with (o,i) on PSUM partitions.
      hm = relu(ea @ Wa + ba)              [1024, e]   (fp8 out, x WA_SCALE)
      thetaT = Wb'.T @ hm                  [(o,i), e]  per 128-row tile
      prod = thetaT * xsrep                (DVE, one pass, bf16 out)
      msgT[o,e] = sum_i prod[(o,i),e]      (PE selector matmul R_t)
    Wb' cols are (o,i)-ordered (col o*fin+i, scaled by FP8_WSCALE); R_t
    carries the 1/(scales) descale. Edge-net bias applied host-side.
    fp8 path: ea/Wa are DoubleRow-packed [8, 2, .] (row 2p+s on
    partition p sub s), Wa scaled by WA_SCALE.
    """
    DW = fin * fout                 # 1024 or 2048
    NT = DW // 128                  # 8 or 16 thetaT tiles
    NB = 5                          # chunks per batched xsrep/msgT DMA
    wdt = _F8 if USE_FP8 else _F32R

    nc = bacc.Bacc(None, target_bir_lowering=False)

    if USE_FP8:
        eaT_d = nc.dram_tensor("eaT", [8, 2 * EP], _F8, kind="ExternalInput")
        Wa_d = nc.dram_tensor("Wa", [8, 2 * D1], _F8, kind="ExternalInput")
    else:
        eaT_d = nc.dram_tensor("eaT", [EF, EP], _F32R, kind="ExternalInput")
        Wa_d = nc.dram_tensor("Wa", [EF, D1], _F32R, kind="ExternalInput")
    xsT_d = nc.dram_tensor("xsT", [fin, EP], _F32, kind="ExternalInput")
    ba_d = nc.dram_tensor("ba", [128, 8], _F32, kind="ExternalInput")
    Wb_d = nc.dram_tensor("Wb", [D1, DW], wdt, kind="ExternalInput")
    R_d = nc.dram_tensor("R", [128, NT * fout], _BF16, kind="ExternalInput")
    msgT_d = nc.dram_tensor("msgT", [fout, EP], _F32, kind="ExternalOutput")

    with tile.TileContext(nc) as tc:
        with (
            tc.tile_pool(name="w", bufs=1) as wpool,
            tc.tile_pool(name="h", bufs=2) as hpool,
            tc.tile_pool(name="xs", bufs=2) as xspool,
            tc.tile_pool(name="pr", bufs=4) as prpool,
            tc.tile_pool(name="o", bufs=2) as opool,
            tc.tile_pool(name="psh", bufs=2, space=bass.MemorySpace.PSUM) as pshpool,
            tc.tile_pool(name="pst", bufs=4, space=bass.MemorySpace.PSUM) as pstpool,
            tc.tile_pool(name="psm", bufs=2, space=bass.MemorySpace.PSUM) as psmpool,
        ):
            if USE_FP8:
                wa = wpool.tile([8, 2 * D1], _F8)
                ea_all = wpool.tile([8, 2 * EP], _F8)
                wa3 = wa[:].rearrange("p (s d) -> p s d", s=2)
                ea3_all = ea_all[:].rearrange("p (s e) -> p s e", s=2)
            else:
                wa = wpool.tile([EF, D1], _F32R)
                ea_all = wpool.tile([EF, EP], _F32R)
            nc.sync.dma_start(wa[:], Wa_d[:])
            nc.sync.dma_start(ea_all[:], eaT_d[:])
            wb = wpool.tile([128, 8 * DW], wdt)
            for k in range(8):
                nc.sync.dma_start(
                    wb[:, k * DW:(k + 1) * DW], Wb_d[k * 128:(k + 1) * 128, :]
                )
            ba = wpool.tile([128, 8], _F32)
            nc.sync.dma_start(ba[:], ba_d[:])
            R = wpool.tile([128, NT * fout], _BF16)
            nc.sync.dma_start(R[:], R_d[:])
            # xsrep_all[p, e] = xs[p % fin, e] for the whole padded edge
            # range: 128//fin plain row-block copies (stride-0 broadcast
            # DMA misbehaves on hw).
            xsrep_all = wpool.tile([128, EP], _F32)
            for r in range(128 // fin):
                nc.sync.dma_start(xsrep_all[r * fin:(r + 1) * fin, :], xsT_d[:])

            wb3 = wb[:].rearrange("p (t d) -> p t d", t=8)

            for c in range(NCHUNK):
                s = c * CH
                if c % NB == 0:
                    msgsb_b = opool.tile([fout, NB * CH], _F32)
                xsrep = xsrep_all[:, s:s + CH]
                xsrep2 = xsrep_all[:].rearrange(
                    "p (h e) -> p h e", h=EP // CH)[
                    :, c, :].unsqueeze(1).broadcast_to([128, 2, CH])

                # hm = relu(Wa.T @ ea + ba): [1024, CH] as 8 tiles
                hm = hpool.tile([128, 8 * CH], wdt)
                for j in range(8):
                    ps = pshpool.tile([128, CH], _F32)
                    if USE_FP8:
                        nc.tensor.matmul(
                            ps[:],
                            wa3[:, :, j * 128:(j + 1) * 128],
                            ea3_all[:, :, s:s + CH],
                            start=True, stop=True,
                            perf_mode=_DR,
                        )
                    else:
                        nc.tensor.matmul(
                            ps[:],
                            wa[:, j * 128:(j + 1) * 128],
                            ea_all[:, s:s + CH],
                            start=True, stop=True,
                        )
                    nc.scalar.activation(
                        hm[:, j * CH:(j + 1) * CH], ps[:], _RELU,
                        bias=ba[:, j:j + 1],
                    )
                hm3 = hm[:].rearrange("p (t e) -> p t e", t=8)

                msgps = psmpool.tile([fout, CH], _F32)
                NP = NT // 2   # theta-tile pairs

                def emit_pair(j):
                    # two theta tiles (2j, 2j+1) into one 2-bank PSUM tile,
                    # one fused scale+mul into fp8 prod for both.
                    P = pstpool.tile([128, 2 * CH], _F32)
                    for h in range(2):
                        t = 2 * j + h
                        if USE_FP8:
                            for g in range(4):
                                nc.tensor.matmul(
                                    P[:, h * CH:(h + 1) * CH],
                                    wb3[:, 2 * g:2 * g + 2, t * 128:(t + 1) * 128],
                                    hm3[:, 2 * g:2 * g + 2, :],
                                    start=(g == 0), stop=(g == 3),
                                    perf_mode=_DR,
                                )
                        else:
                            for k in range(8):
                                nc.tensor.matmul(
                                    P[:, h * CH:(h + 1) * CH],
                                    wb3[:, k, t * 128:(t + 1) * 128],
                                    hm3[:, k, :],
                                    start=(k == 0), stop=(k == 7),
                                )
                    prod = prpool.tile([128, 2 * CH], _F8 if USE_FP8 else _BF16)
                    nc.vector.scalar_tensor_tensor(
                        prod[:], P[:], THETA_DESCALE,
                        xsrep2, _MUL, _MUL,
                    )
                    return prod

                def emit_reduce(j, prod):
                    if USE_FP8:
                        # DoubleRow: sub s = theta tile 2j+s
                        nc.tensor.matmul(
                            msgps[:],
                            R[:, 2 * j * fout:(2 * j + 2) * fout]
                            .rearrange("p (s o) -> p s o", s=2),
                            prod[:].rearrange("p (s e) -> p s e", s=2),
                            start=(j == 0), stop=(j == NP - 1),
                            perf_mode=_DR,
                        )
                    else:
                        for h in range(2):
                            t = 2 * j + h
                            nc.tensor.matmul(
                                msgps[:], R[:, t * fout:(t + 1) * fout],
                                prod[:, h * CH:(h + 1) * CH],
                                start=(t == 0), stop=(t == NT - 1),
                            )

                # software-pipeline: keep mains ahead of each reduce so the
                # PE never head-of-line blocks on the DVE.
                prods = {}
                for j in range(NP):
                    prods[j] = emit_pair(j)
                    if j >= 1:
                        emit_reduce(j - 1, prods.pop(j - 1))
                emit_reduce(NP - 1, prods.pop(NP - 1))
                nc.scalar.copy(
                    msgsb_b[:, (c % NB) * CH:(c % NB + 1) * CH], msgps[:])
                if c % NB == NB - 1:
                    nc.sync.dma_start(
                        msgT_d[:, (c - NB + 1) * CH:(c + 1) * CH], msgsb_b[:])

    nc.compile()
    return nc


def _get_nc(tag):
    if tag not in _NC_CACHE:
        fin, fout = (F_IN, H) if tag == "A" else (H, H2)
        _NC_CACHE[tag] = _build_layer(tag, fin, fout)
    return _NC_CACHE[tag]


def compiled_ncs():
    return [_get_nc("A"), _get_nc("B")]


def _relu(v):
    return np.maximum(v, 0.0)


def _segmean(vals, idx, n):
    s = np.zeros((n, vals.shape[1]), np.float32)
    np.add.at(s, idx, vals)
    c = np.bincount(idx, minlength=n).astype(np.float32)
    return s / np.maximum(c, 1.0)[:, None]


def _reorder_oi(Wb, fin, fout):
    """Reorder edge-net output cols from (i, o) = i*fout + o to (o, i) =
    o*fin + i, so each 512-col block is 16 complete outputs."""
    W = Wb.reshape(D1, fin, fout).transpose(0, 2, 1).reshape(D1, fin * fout)
    return np.ascontiguousarray(W)


def _run_layer(tag, ea, feat_src, Wa, ba, Wb_r, bb, fout):
    """ea: [E,16] f32; feat_src: [E, fin] f32 (features gathered at src).
    bb: raw edge-net output bias [fin*fout] - applied host-side as
    msg += feat_src @ bb.reshape(fin, fout)."""
    import ml_dtypes
    fin = feat_src.shape[1]
    nc = _get_nc(tag)
    if USE_FP8:
        baT = np.ascontiguousarray((ba * WA_SCALE).reshape(8, 128).T)
        Wb_fin = (Wb_r * FP8_WSCALE).astype(ml_dtypes.float8_e4m3)
        Wa_fin = np.ascontiguousarray(
            (Wa * WA_SCALE).reshape(8, 2, D1)).astype(ml_dtypes.float8_e4m3)
        descale = 1.0 / (FP8_WSCALE * WA_SCALE)
    else:
        baT = np.ascontiguousarray(ba.reshape(8, 128).T)
        Wb_fin = Wb_r
        Wa_fin = Wa
        descale = 1.0
    # R_t[p, o] = (o == t*(128//fin) + p//fin) * descale, stacked over t
    NT = (fin * fout) // 128
    R = np.zeros((128, NT * fout), np.float32)
    for t in range(NT):
        for p in range(128):
            R[p, t * fout + t * (128 // fin) + p // fin] = descale
    R = R.astype(ml_dtypes.bfloat16)
    in_maps = []
    for i in range(NC):
        eaT = np.zeros((EF, EP), np.float32)
        eaT[:, :EPC] = ea[i * EPC:(i + 1) * EPC].T
        if USE_FP8:
            eaT = np.ascontiguousarray(
                eaT.reshape(8, 2, EP)).astype(ml_dtypes.float8_e4m3)
        xsT = np.zeros((fin, EP), np.float32)
        xsT[:, :EPC] = feat_src[i * EPC:(i + 1) * EPC].T
        in_maps.append(dict(eaT=eaT, xsT=xsT, Wa=Wa_fin, ba=baT, Wb=Wb_fin, R=R))
    res = bass_utils.run_bass_kernel_spmd(nc, in_maps, core_ids=list(range(NC)))
    msg = np.concatenate(
        [res.results[i]["msgT"][:, :EPC].T for i in range(NC)], axis=0)
    return msg + feat_src @ bb.reshape(fin, fout)


def kernel(**inputs):
    x = np.asarray(inputs["x"], np.float32)
    edge_index = np.asarray(inputs["edge_index"])
    eap = np.asarray(inputs["edge_attr_packed"])
    batch = np.asarray(inputs["batch"])
    W1a = np.ascontiguousarray(inputs["W1a"], np.float32)
    W1b = np.ascontiguousarray(inputs["W1b"], np.float32)
    W2a = np.ascontiguousarray(inputs["W2a"], np.float32)
    W2b = np.ascontiguousarray(inputs["W2b"], np.float32)
    b1a = np.asarray(inputs["b1a"], np.float32)
    b1b = np.asarray(inputs["b1b"], np.float32)
    b2a = np.asarray(inputs["b2a"], np.float32)
    b2b = np.asarray(inputs["b2b"], np.float32)
    root1 = np.asarray(inputs["root1"], np.float32)
    bias1 = np.asarray(inputs["bias1"], np.float32)
    root2 = np.asarray(inputs["root2"], np.float32)
    bias2 = np.asarray(inputs["bias2"], np.float32)

    # MSB-first bit unpack -> [E, 16]
    shifts = np.arange(7, -1, -1, dtype=np.int32)
    ea = ((eap[:, :, None].astype(np.int32) >> shifts) & 1).reshape(E, -1)
    ea = ea.astype(np.float32)

    src, dst = edge_index[0], edge_index[1]

    W1b_r = _reorder_oi(W1b, F_IN, H)
    msg1 = _run_layer("A", ea, x[src], W1a, b1a, W1b_r, b1b, H)
    h = _relu(_segmean(msg1, dst, N) + x @ root1 + bias1)

    W2b_r = _reorder_oi(W2b, H, H2)
    msg2 = _run_layer("B", ea, h[src], W2a, b2a, W2b_r, b2b, H2)
    h = _relu(_segmean(msg2, dst, N) + h @ root2 + bias2)

    g = _segmean(h, batch, NG)
    g = _relu(g @ np.asarray(inputs["fcW1"], np.float32) + np.asarray(inputs["fcb1"], np.float32))
    g = _relu(g @ np.asarray(inputs["fcW2"], np.float32) + np.asarray(inputs["fcb2"], np.float32))
    g = _relu(g @ np.asarray(inputs["fcW3"], np.float32) + np.asarray(inputs["fcb3"], np.float32))
    return (g @ np.asarray(inputs["fcW4"], np.float32) + np.asarray(inputs["fcb4"], np.float32)).astype(np.float32)


# revision 26
# speedup vs baseline: 9.4291x; 1.1698x over previous
"""GCN-with-edge-features kernel for 8 Trainium2 cores.

Data-parallel over edges (12500/core, padded to 12800). Two device
launches:
  A: h1 = relu(ea@W1a+b1a); theta1 = h1@W1b+b1b (kept in PSUM);
     msg1[e,o] = sum_i x[src[e],i] * theta1[e,i,o]   -> [E, 32]
  B: same with W2a/W2b and h[src]                    -> [E, 64]
Matmuls run as float32r (1 cycle/row). theta never leaves the chip;
the per-edge contraction is a DVE strided multiply + reduce with edges
on partitions. Host does only the segment-mean scatters, the tiny
root/bias adds, and the final 2000-row MLP.
"""
import numpy as np

import sys
for p in ("/opt/trn_rl_repo",):
    if p not in sys.path:
        sys.path.append(p)

from concourse import bass, bacc, mybir, tile
from concourse import bass_utils

E = 100000
N = 50000
NG = 2000
F_IN = 32
EF = 16
H = 32
H2 = 64
NC = 8
EPC = E // NC          # 12500 edges per core
CH = 512
NCHUNK = 25
EP = CH * NCHUNK       # 12800 padded edges per core
EB = CH // 128         # 4 edge blocks per chunk
D1 = H * F_IN          # 1024
D2 = H * H2            # 2048

_F32 = mybir.dt.float32
_F32R = mybir.dt.float32r
_F8 = mybir.dt.float8e4
_BF16 = mybir.dt.bfloat16
_DR = mybir.MatmulPerfMode.DoubleRow

USE_FP8 = True          # fp8e4m3 + DoubleRow for the big GEMMs
FP8_WSCALE = 64.0       # Wb pre-scale (W ~ +-1/32 underflows e4m3)
WA_SCALE = 16.0         # Wa pre-scale for the fp8 edge-net input GEMM
MUL_ENGINE = "vector"   # engine for the per-edge multiply
_RELU = mybir.ActivationFunctionType.Relu
_AX_X = mybir.AxisListType.X
_MUL = mybir.AluOpType.mult
_ADD = mybir.AluOpType.add

_NC_CACHE = {}


def _build_layer(tag, fin, fout):
    """One NNConv layer program: edge-net MLP + per-edge contraction.

    Orientation: theta^T [(o,i), e] with (o,i) on PSUM partitions.
      hm = relu(ea @ Wa + ba)              [1024, e]   (fp8 out, x WA_SCALE)
      thetaT = Wb'.T @ hm                  [(o,i), e]  per 128-row tile
      prod = thetaT * xsrep                (DVE, one pass, bf16 out)
      msgT[o,e] = sum_i prod[(o,i),e]      (PE selector matmul R_t)
    Wb' cols are (o,i)-ordered (col o*fin+i, scaled by FP8_WSCALE); R_t
    carries the 1/(scales) descale. Edge-net bias applied host-side.
    fp8 path: ea/Wa are DoubleRow-packed [8, 2, .] (row 2p+s on
    partition p sub s), Wa scaled by WA_SCALE.
    """
    DW = fin * fout                 # 1024 or 2048
    NT = DW // 128                  # 8 or 16 thetaT tiles
    NB = 5                          # chunks per batched xsrep/msgT DMA
    wdt = _F8 if USE_FP8 else _F32R
    descale = 1.0 / (FP8_WSCALE * WA_SCALE) if USE_FP8 else 1.0

    nc = bacc.Bacc(None, target_bir_lowering=False)

    if USE_FP8:
        eaT_d = nc.dram_tensor("eaT", [8, 2 * EP], _F8, kind="ExternalInput")
        Wa_d = nc.dram_tensor("Wa", [8, 2 * D1], _F8, kind="ExternalInput")
    else:
        eaT_d = nc.dram_tensor("eaT", [EF, EP], _F32R, kind="ExternalInput")
        Wa_d = nc.dram_tensor("Wa", [EF, D1], _F32R, kind="ExternalInput")
    xsT_d = nc.dram_tensor("xsT", [fin, EP], _F32, kind="ExternalInput")
    ba_d = nc.dram_tensor("ba", [128, 8], _F32, kind="ExternalInput")
    Wb_d = nc.dram_tensor("Wb", [D1, DW], wdt, kind="ExternalInput")
    rdt = _F8 if USE_FP8 else _BF16
    R_d = nc.dram_tensor("R", [128, NT * fout], rdt, kind="ExternalInput")
    msgT_d = nc.dram_tensor("msgT", [fout, EP], _F32, kind="ExternalOutput")

    with tile.TileContext(nc) as tc:
        with (
            tc.tile_pool(name="w", bufs=1) as wpool,
            tc.tile_pool(name="h", bufs=2) as hpool,
            tc.tile_pool(name="xs", bufs=2) as xspool,
            tc.tile_pool(name="pr", bufs=4) as prpool,
            tc.tile_pool(name="o", bufs=2) as opool,
            tc.tile_pool(name="psh", bufs=1, space=bass.MemorySpace.PSUM) as pshpool,
            tc.tile_pool(name="pst", bufs=3, space=bass.MemorySpace.PSUM) as pstpool,
            tc.tile_pool(name="psm", bufs=1, space=bass.MemorySpace.PSUM) as psmpool,
        ):
            if USE_FP8:
                wa = wpool.tile([8, 2 * D1], _F8)
                ea_all = wpool.tile([8, 2 * EP], _F8)
                wa3 = wa[:].rearrange("p (s d) -> p s d", s=2)
                ea3_all = ea_all[:].rearrange("p (s e) -> p s e", s=2)
            else:
                wa = wpool.tile([EF, D1], _F32R)
                ea_all = wpool.tile([EF, EP], _F32R)
            nc.sync.dma_start(wa[:], Wa_d[:])
            nc.sync.dma_start(ea_all[:], eaT_d[:])
            wb = wpool.tile([128, 8 * DW], wdt)
            for k in range(8):
                nc.sync.dma_start(
                    wb[:, k * DW:(k + 1) * DW], Wb_d[k * 128:(k + 1) * 128, :]
                )
            ba = wpool.tile([128, 8], _F32)
            nc.sync.dma_start(ba[:], ba_d[:])
            R = wpool.tile([128, NT * fout], rdt)
            nc.sync.dma_start(R[:], R_d[:])
            # xsrep_all[p, e] = xs[p % fin, e] for the whole padded edge
            # range: 128//fin plain row-block copies (stride-0 broadcast
            # DMA misbehaves on hw).
            xsrep_all = wpool.tile([128, EP], _F32)
            for r in range(128 // fin):
                nc.sync.dma_start(xsrep_all[r * fin:(r + 1) * fin, :], xsT_d[:])

            wb3 = wb[:].rearrange("p (t d) -> p t d", t=8)

            for c in range(NCHUNK):
                s = c * CH
                if c % NB == 0:
                    msgsb_b = opool.tile([fout, NB * CH], _F32)
                xsrep = xsrep_all[:, s:s + CH]

                # hm = relu(Wa.T @ ea + ba): [1024, CH] as 8 tiles
                hm = hpool.tile([128, 8 * CH], wdt)
                for j in range(8):
                    ps = pshpool.tile([128, CH], _F32)
                    if USE_FP8:
                        nc.tensor.matmul(
                            ps[:],
                            wa3[:, :, j * 128:(j + 1) * 128],
                            ea3_all[:, :, s:s + CH],
                            start=True, stop=True,
                            perf_mode=_DR,
                        )
                    else:
                        nc.tensor.matmul(
                            ps[:],
                            wa[:, j * 128:(j + 1) * 128],
                            ea_all[:, s:s + CH],
                            start=True, stop=True,
                        )
                    nc.scalar.activation(
                        hm[:, j * CH:(j + 1) * CH], ps[:], _RELU,
                        bias=ba[:, j:j + 1],
                    )
                hm3 = hm[:].rearrange("p (t e) -> p t e", t=8)

                msgps = psmpool.tile([fout, CH], _F32)
                NP = NT // 2   # theta-tile pairs

                def emit_pair(j):
                    # two theta tiles (2j, 2j+1) into one 2-bank PSUM tile,
                    # one fused scale+mul into fp8 prod for both.
                    P = pstpool.tile([128, 2 * CH], _F32)
                    for h in range(2):
                        t = 2 * j + h
                        if USE_FP8:
                            for g in range(4):
                                nc.tensor.matmul(
                                    P[:, h * CH:(h + 1) * CH],
                                    wb3[:, 2 * g:2 * g + 2, t * 128:(t + 1) * 128],
                                    hm3[:, 2 * g:2 * g + 2, :],
                                    start=(g == 0), stop=(g == 3),
                                    perf_mode=_DR,
                                )
                        else:
                            for k in range(8):
                                nc.tensor.matmul(
                                    P[:, h * CH:(h + 1) * CH],
                                    wb3[:, k, t * 128:(t + 1) * 128],
                                    hm3[:, k, :],
                                    start=(k == 0), stop=(k == 7),
                                )
                    prod = prpool.tile([128, 2 * CH], _F8 if USE_FP8 else _BF16)
                    nc.vector.scalar_tensor_tensor(
                        prod[:].rearrange("p (s e) -> p s e", s=2),
                        P[:].rearrange("p (s e) -> p s e", s=2),
                        descale,
                        xsrep.unsqueeze(1).broadcast_to([128, 2, CH]),
                        _MUL, _MUL,
                    )
                    return prod

                def emit_reduce(j, prod):
                    if USE_FP8:
                        # DoubleRow: sub s = theta tile 2j+s
                        nc.tensor.matmul(
                            msgps[:],
                            R[:, 2 * j * fout:(2 * j + 2) * fout]
                            .rearrange("p (s o) -> p s o", s=2),
                            prod[:].rearrange("p (s e) -> p s e", s=2),
                            start=(j == 0), stop=(j == NP - 1),
                            perf_mode=_DR,
                        )
                    else:
                        for h in range(2):
                            t = 2 * j + h
                            nc.tensor.matmul(
                                msgps[:], R[:, t * fout:(t + 1) * fout],
                                prod[:, h * CH:(h + 1) * CH],
                                start=(t == 0), stop=(t == NT - 1),
                            )

                # software-pipeline: keep mains ahead of each reduce so the
                # PE never head-of-line blocks on the DVE.
                prods = {}
                for j in range(NP):
                    prods[j] = emit_pair(j)
                    if j >= 1:
                        emit_reduce(j - 1, prods.pop(j - 1))
                emit_reduce(NP - 1, prods.pop(NP - 1))
                nc.scalar.copy(
                    msgsb_b[:, (c % NB) * CH:(c % NB + 1) * CH], msgps[:])
                if c % NB == NB - 1:
                    nc.sync.dma_start(
                        msgT_d[:, (c - NB + 1) * CH:(c + 1) * CH], msgsb_b[:])

    nc.compile()
    return nc


def _get_nc(tag):
    if tag not in _NC_CACHE:
        fin, fout = (F_IN, H) if tag == "A" else (H, H2)
        _NC_CACHE[tag] = _build_layer(tag, fin, fout)
    return _NC_CACHE[tag]


def compiled_ncs():
    return [_get_nc("A"), _get_nc("B")]


def _relu(v):
    return np.maximum(v, 0.0)


def _segmean(vals, idx, n):
    s = np.zeros((n, vals.shape[1]), np.float32)
    np.add.at(s, idx, vals)
    c = np.bincount(idx, minlength=n).astype(np.float32)
    return s / np.maximum(c, 1.0)[:, None]


def _reorder_oi(Wb, fin, fout):
    """Reorder edge-net output cols from (i, o) = i*fout + o to (o, i) =
    o*fin + i, so each 512-col block is 16 complete outputs."""
    W = Wb.reshape(D1, fin, fout).transpose(0, 2, 1).reshape(D1, fin * fout)
    return np.ascontiguousarray(W)


def _run_layer(tag, ea, feat_src, Wa, ba, Wb_r, bb, fout):
    """ea: [E,16] f32; feat_src: [E, fin] f32 (features gathered at src).
    bb: raw edge-net output bias [fin*fout] - applied host-side as
    msg += feat_src @ bb.reshape(fin, fout)."""
    import ml_dtypes
    fin = feat_src.shape[1]
    nc = _get_nc(tag)
    if USE_FP8:
        baT = np.ascontiguousarray((ba * WA_SCALE).reshape(8, 128).T)
        Wb_fin = (Wb_r * FP8_WSCALE).astype(ml_dtypes.float8_e4m3)
        Wa_fin = np.ascontiguousarray(
            (Wa * WA_SCALE).reshape(8, 2, D1)).astype(ml_dtypes.float8_e4m3)
    else:
        baT = np.ascontiguousarray(ba.reshape(8, 128).T)
        Wb_fin = Wb_r
        Wa_fin = Wa
    # R_t[p, o] = (o == t*(128//fin) + p//fin), stacked over t (the fp8
    # descale is applied inside the device stt op)
    NT = (fin * fout) // 128
    R = np.zeros((128, NT * fout), np.float32)
    for t in range(NT):
        for p in range(128):
            R[p, t * fout + t * (128 // fin) + p // fin] = 1.0
    R = R.astype(ml_dtypes.float8_e4m3 if USE_FP8 else ml_dtypes.bfloat16)
    in_maps = []
    for i in range(NC):
        eaT = np.zeros((EF, EP), np.float32)
        eaT[:, :EPC] = ea[i * EPC:(i + 1) * EPC].T
        if USE_FP8:
            eaT = np.ascontiguousarray(
                eaT.reshape(8, 2, EP)).astype(ml_dtypes.float8_e4m3)
        xsT = np.zeros((fin, EP), np.float32)
        xsT[:, :EPC] = feat_src[i * EPC:(i + 1) * EPC].T
        in_maps.append(dict(eaT=eaT, xsT=xsT, Wa=Wa_fin, ba=baT, Wb=Wb_fin, R=R))
    res = bass_utils.run_bass_kernel_spmd(nc, in_maps, core_ids=list(range(NC)))
    msg = np.concatenate(
        [res.results[i]["msgT"][:, :EPC].T for i in range(NC)], axis=0)
    return msg + feat_src @ bb.reshape(fin, fout)


def kernel(**inputs):
    x = np.asarray(inputs["x"], np.float32)
    edge_index = np.asarray(inputs["edge_index"])
    eap = np.asarray(inputs["edge_attr_packed"])
    batch = np.asarray(inputs["batch"])
    W1a = np.ascontiguousarray(inputs["W1a"], np.float32)
    W1b = np.ascontiguousarray(inputs["W1b"], np.float32)
    W2a = np.ascontiguousarray(inputs["W2a"], np.float32)
    W2b = np.ascontiguousarray(inputs["W2b"], np.float32)
    b1a = np.asarray(inputs["b1a"], np.float32)
    b1b = np.asarray(inputs["b1b"], np.float32)
    b2a = np.asarray(inputs["b2a"], np.float32)
    b2b = np.asarray(inputs["b2b"], np.float32)
    root1 = np.asarray(inputs["root1"], np.float32)
    bias1 = np.asarray(inputs["bias1"], np.float32)
    root2 = np.asarray(inputs["root2"], np.float32)
    bias2 = np.asarray(inputs["bias2"], np.float32)

    # MSB-first bit unpack -> [E, 16]
    shifts = np.arange(7, -1, -1, dtype=np.int32)
    ea = ((eap[:, :, None].astype(np.int32) >> shifts) & 1).reshape(E, -1)
    ea = ea.astype(np.float32)

    src, dst = edge_index[0], edge_index[1]

    W1b_r = _reorder_oi(W1b, F_IN, H)
    msg1 = _run_layer("A", ea, x[src], W1a, b1a, W1b_r, b1b, H)
    h = _relu(_segmean(msg1, dst, N) + x @ root1 + bias1)

    W2b_r = _reorder_oi(W2b, H, H2)
    msg2 = _run_layer("B", ea, h[src], W2a, b2a, W2b_r, b2b, H2)
    h = _relu(_segmean(msg2, dst, N) + h @ root2 + bias2)

    g = _segmean(h, batch, NG)
    g = _relu(g @ np.asarray(inputs["fcW1"], np.float32) + np.asarray(inputs["fcb1"], np.float32))
    g = _relu(g @ np.asarray(inputs["fcW2"], np.float32) + np.asarray(inputs["fcb2"], np.float32))
    g = _relu(g @ np.asarray(inputs["fcW3"], np.float32) + np.asarray(inputs["fcb3"], np.float32))
    return (g @ np.asarray(inputs["fcW4"], np.float32) + np.asarray(inputs["fcb4"], np.float32)).astype(np.float32)


# revision 31
# speedup vs baseline: 9.8459x; 1.0442x over previous
"""GCN-with-edge-features kernel for 8 Trainium2 cores.

Data-parallel over edges (12500/core, padded to 12800). Two device
launches:
  A: h1 = relu(ea@W1a+b1a); theta1 = h1@W1b+b1b (kept in PSUM);
     msg1[e,o] = sum_i x[src[e],i] * theta1[e,i,o]   -> [E, 32]
  B: same with W2a/W2b and h[src]                    -> [E, 64]
Matmuls run as float32r (1 cycle/row). theta never leaves the chip;
the per-edge contraction is a DVE strided multiply + reduce with edges
on partitions. Host does only the segment-mean scatters, the tiny
root/bias adds, and the final 2000-row MLP.
"""
import numpy as np

import sys
for p in ("/opt/trn_rl_repo",):
    if p not in sys.path:
        sys.path.append(p)

from concourse import bass, bacc, mybir, tile
from concourse import bass_utils

E = 100000
N = 50000
NG = 2000
F_IN = 32
EF = 16
H = 32
H2 = 64
NC = 8
EPC = E // NC          # 12500 edges per core
CH = 512
NCHUNK = 25
EP = CH * NCHUNK       # 12800 padded edges per core
EB = CH // 128         # 4 edge blocks per chunk
D1 = H * F_IN          # 1024
D2 = H * H2            # 2048

_F32 = mybir.dt.float32
_F32R = mybir.dt.float32r
_F8 = mybir.dt.float8e4
_BF16 = mybir.dt.bfloat16
_DR = mybir.MatmulPerfMode.DoubleRow

USE_FP8 = True          # fp8e4m3 + DoubleRow for the big GEMMs
FP8_WSCALE = 64.0       # Wb pre-scale (W ~ +-1/32 underflows e4m3)
WA_SCALE = 16.0         # Wa pre-scale for the fp8 edge-net input GEMM
MUL_ENGINE = "vector"   # engine for the per-edge multiply
_RELU = mybir.ActivationFunctionType.Relu
_AX_X = mybir.AxisListType.X
_MUL = mybir.AluOpType.mult
_ADD = mybir.AluOpType.add

_NC_CACHE = {}


def _build_layer(tag, fin, fout):
    """One NNConv layer program: edge-net MLP + per-edge contraction.

    Orientation: theta^T [(o,i), e] with (o,i) on PSUM partitions.
      hm = relu(ea @ Wa + ba)              [1024, e]   (fp8 out, x WA_SCALE)
      thetaT = Wb'.T @ hm                  [(o,i), e]  per 128-row tile
      prod = thetaT * xsrep                (DVE, one pass, bf16 out)
      msgT[o,e] = sum_i prod[(o,i),e]      (PE selector matmul R_t)
    Wb' cols are (o,i)-ordered (col o*fin+i, scaled by FP8_WSCALE); R_t
    carries the 1/(scales) descale. Edge-net bias applied host-side.
    fp8 path: ea/Wa are DoubleRow-packed [8, 2, .] (row 2p+s on
    partition p sub s), Wa scaled by WA_SCALE.
    """
    DW = fin * fout                 # 1024 or 2048
    NT = DW // 128                  # 8 or 16 thetaT tiles
    NB = 5                          # chunks per batched xsrep/msgT DMA
    wdt = _F8 if USE_FP8 else _F32R
    descale = 1.0 / (FP8_WSCALE * WA_SCALE) if USE_FP8 else 1.0

    nc = bacc.Bacc(None, target_bir_lowering=False)

    if USE_FP8:
        eaT_d = nc.dram_tensor("eaT", [8, 2 * EP], _F8, kind="ExternalInput")
        Wa_d = nc.dram_tensor("Wa", [8, 2 * D1], _F8, kind="ExternalInput")
    else:
        eaT_d = nc.dram_tensor("eaT", [EF, EP], _F32R, kind="ExternalInput")
        Wa_d = nc.dram_tensor("Wa", [EF, D1], _F32R, kind="ExternalInput")
    xsT_d = nc.dram_tensor("xsT", [fin, EP], _BF16, kind="ExternalInput")
    ba_d = nc.dram_tensor("ba", [128, 8], _F32, kind="ExternalInput")
    Wb_d = nc.dram_tensor("Wb", [D1, DW], wdt, kind="ExternalInput")
    rdt = _F8 if USE_FP8 else _BF16
    R_d = nc.dram_tensor("R", [128, NT * fout], rdt, kind="ExternalInput")
    msgT_d = nc.dram_tensor("msgT", [fout, EP], _F32, kind="ExternalOutput")

    psh_b, pst_b, psm_b = (2, 2, 2) if tag == "A" else (1, 3, 1)
    with tile.TileContext(nc) as tc:
        with (
            tc.tile_pool(name="w", bufs=1) as wpool,
            tc.tile_pool(name="h", bufs=2) as hpool,
            tc.tile_pool(name="xs", bufs=2) as xspool,
            tc.tile_pool(name="pr", bufs=4) as prpool,
            tc.tile_pool(name="o", bufs=2) as opool,
            tc.tile_pool(name="psh", bufs=psh_b, space=bass.MemorySpace.PSUM) as pshpool,
            tc.tile_pool(name="pst", bufs=pst_b, space=bass.MemorySpace.PSUM) as pstpool,
            tc.tile_pool(name="psm", bufs=psm_b, space=bass.MemorySpace.PSUM) as psmpool,
        ):
            if USE_FP8:
                wa = wpool.tile([8, 2 * D1], _F8)
                ea_all = wpool.tile([8, 2 * EP], _F8)
                wa3 = wa[:].rearrange("p (s d) -> p s d", s=2)
                ea3_all = ea_all[:].rearrange("p (s e) -> p s e", s=2)
            else:
                wa = wpool.tile([EF, D1], _F32R)
                ea_all = wpool.tile([EF, EP], _F32R)
            nc.sync.dma_start(wa[:], Wa_d[:])
            nc.sync.dma_start(ea_all[:], eaT_d[:])
            ba = wpool.tile([128, 8], _F32)
            nc.sync.dma_start(ba[:], ba_d[:])
            # xsrep_all[p, e] = xs[p % fin, e] for the whole padded edge
            # range: 128//fin plain row-block copies (stride-0 broadcast
            # DMA misbehaves on hw). bf16 to halve the prologue DMA.
            xsrep_all = wpool.tile([128, EP], _BF16)
            for r in range(128 // fin):
                nc.sync.dma_start(xsrep_all[r * fin:(r + 1) * fin, :], xsT_d[:])
            wb = wpool.tile([128, 8 * DW], wdt)
            for k in range(8):
                nc.sync.dma_start(
                    wb[:, k * DW:(k + 1) * DW], Wb_d[k * 128:(k + 1) * 128, :]
                )
            R = wpool.tile([128, NT * fout], rdt)
            nc.sync.dma_start(R[:], R_d[:])

            wb3 = wb[:].rearrange("p (t d) -> p t d", t=8)

            def emit_layer_a(c):
                # hm = relu(Wa.T @ ea + ba): [1024, CH] as 8 tiles
                s = c * CH
                hm = hpool.tile([128, 8 * CH], wdt)
                for j in range(8):
                    ps = pshpool.tile([128, CH], _F32)
                    if USE_FP8:
                        nc.tensor.matmul(
                            ps[:],
                            wa3[:, :, j * 128:(j + 1) * 128],
                            ea3_all[:, :, s:s + CH],
                            start=True, stop=True,
                            perf_mode=_DR,
                        )
                    else:
                        nc.tensor.matmul(
                            ps[:],
                            wa[:, j * 128:(j + 1) * 128],
                            ea_all[:, s:s + CH],
                            start=True, stop=True,
                        )
                    nc.scalar.activation(
                        hm[:, j * CH:(j + 1) * CH], ps[:], _RELU,
                        bias=ba[:, j:j + 1],
                    )
                return hm

            hm_next = emit_layer_a(0)
            for c in range(NCHUNK):
                s = c * CH
                if c % NB == 0:
                    msgsb_b = opool.tile([fout, NB * CH], _F32)
                xsrep = xsrep_all[:, s:s + CH]

                hm = hm_next
                if c + 1 < NCHUNK:
                    hm_next = emit_layer_a(c + 1)
                hm3 = hm[:].rearrange("p (t e) -> p t e", t=8)

                msgps = psmpool.tile([fout, CH], _F32)
                NP = NT // 2   # theta-tile pairs

                def emit_pair(j):
                    # two theta tiles (2j, 2j+1) into one 2-bank PSUM tile,
                    # one fused scale+mul into fp8 prod for both.
                    P = pstpool.tile([128, 2 * CH], _F32)
                    for h in range(2):
                        t = 2 * j + h
                        if USE_FP8:
                            for g in range(4):
                                nc.tensor.matmul(
                                    P[:, h * CH:(h + 1) * CH],
                                    wb3[:, 2 * g:2 * g + 2, t * 128:(t + 1) * 128],
                                    hm3[:, 2 * g:2 * g + 2, :],
                                    start=(g == 0), stop=(g == 3),
                                    perf_mode=_DR,
                                )
                        else:
                            for k in range(8):
                                nc.tensor.matmul(
                                    P[:, h * CH:(h + 1) * CH],
                                    wb3[:, k, t * 128:(t + 1) * 128],
                                    hm3[:, k, :],
                                    start=(k == 0), stop=(k == 7),
                                )
                    prod = prpool.tile([128, 2 * CH], _F8 if USE_FP8 else _BF16)
                    nc.vector.scalar_tensor_tensor(
                        prod[:].rearrange("p (s e) -> p s e", s=2),
                        P[:].rearrange("p (s e) -> p s e", s=2),
                        descale,
                        xsrep.unsqueeze(1).broadcast_to([128, 2, CH]),
                        _MUL, _MUL,
                    )
                    return prod

                def emit_reduce(j, prod):
                    if USE_FP8:
                        # DoubleRow: sub s = theta tile 2j+s
                        nc.tensor.matmul(
                            msgps[:],
                            R[:, 2 * j * fout:(2 * j + 2) * fout]
                            .rearrange("p (s o) -> p s o", s=2),
                            prod[:].rearrange("p (s e) -> p s e", s=2),
                            start=(j == 0), stop=(j == NP - 1),
                            perf_mode=_DR,
                        )
                    else:
                        for h in range(2):
                            t = 2 * j + h
                            nc.tensor.matmul(
                                msgps[:], R[:, t * fout:(t + 1) * fout],
                                prod[:, h * CH:(h + 1) * CH],
                                start=(t == 0), stop=(t == NT - 1),
                            )

                # software-pipeline: keep mains ahead of each reduce so the
                # PE never head-of-line blocks on the DVE.
                prods = {}
                for j in range(NP):
                    prods[j] = emit_pair(j)
                    if j >= 1:
                        emit_reduce(j - 1, prods.pop(j - 1))
                emit_reduce(NP - 1, prods.pop(NP - 1))
                nc.scalar.copy(
                    msgsb_b[:, (c % NB) * CH:(c % NB + 1) * CH], msgps[:])
                if c % NB == NB - 1:
                    nc.sync.dma_start(
                        msgT_d[:, (c - NB + 1) * CH:(c + 1) * CH], msgsb_b[:])

    nc.compile()
    return nc


def _get_nc(tag):
    if tag not in _NC_CACHE:
        fin, fout = (F_IN, H) if tag == "A" else (H, H2)
        _NC_CACHE[tag] = _build_layer(tag, fin, fout)
    return _NC_CACHE[tag]


def compiled_ncs():
    return [_get_nc("A"), _get_nc("B")]


def _relu(v):
    return np.maximum(v, 0.0)


def _segmean(vals, idx, n):
    s = np.zeros((n, vals.shape[1]), np.float32)
    np.add.at(s, idx, vals)
    c = np.bincount(idx, minlength=n).astype(np.float32)
    return s / np.maximum(c, 1.0)[:, None]


def _reorder_oi(Wb, fin, fout):
    """Reorder edge-net output cols from (i, o) = i*fout + o to (o, i) =
    o*fin + i, so each 512-col block is 16 complete outputs."""
    W = Wb.reshape(D1, fin, fout).transpose(0, 2, 1).reshape(D1, fin * fout)
    return np.ascontiguousarray(W)


def _run_layer(tag, ea, feat_src, Wa, ba, Wb_r, bb, fout):
    """ea: [E,16] f32; feat_src: [E, fin] f32 (features gathered at src).
    bb: raw edge-net output bias [fin*fout] - applied host-side as
    msg += feat_src @ bb.reshape(fin, fout)."""
    import ml_dtypes
    fin = feat_src.shape[1]
    nc = _get_nc(tag)
    if USE_FP8:
        baT = np.ascontiguousarray((ba * WA_SCALE).reshape(8, 128).T)
        Wb_fin = (Wb_r * FP8_WSCALE).astype(ml_dtypes.float8_e4m3)
        Wa_fin = np.ascontiguousarray(
            (Wa * WA_SCALE).reshape(8, 2, D1)).astype(ml_dtypes.float8_e4m3)
    else:
        baT = np.ascontiguousarray(ba.reshape(8, 128).T)
        Wb_fin = Wb_r
        Wa_fin = Wa
    # R_t[p, o] = (o == t*(128//fin) + p//fin), stacked over t (the fp8
    # descale is applied inside the device stt op)
    NT = (fin * fout) // 128
    R = np.zeros((128, NT * fout), np.float32)
    for t in range(NT):
        for p in range(128):
            R[p, t * fout + t * (128 // fin) + p // fin] = 1.0
    R = R.astype(ml_dtypes.float8_e4m3 if USE_FP8 else ml_dtypes.bfloat16)
    in_maps = []
    for i in range(NC):
        eaT = np.zeros((EF, EP), np.float32)
        eaT[:, :EPC] = ea[i * EPC:(i + 1) * EPC].T
        if USE_FP8:
            eaT = np.ascontiguousarray(
                eaT.reshape(8, 2, EP)).astype(ml_dtypes.float8_e4m3)
        xsT = np.zeros((fin, EP), np.float32)
        xsT[:, :EPC] = feat_src[i * EPC:(i + 1) * EPC].T
        xsT = xsT.astype(ml_dtypes.bfloat16)
        in_maps.append(dict(eaT=eaT, xsT=xsT, Wa=Wa_fin, ba=baT, Wb=Wb_fin, R=R))
    res = bass_utils.run_bass_kernel_spmd(nc, in_maps, core_ids=list(range(NC)))
    msg = np.concatenate(
        [res.results[i]["msgT"][:, :EPC].T for i in range(NC)], axis=0)
    return msg + feat_src @ bb.reshape(fin, fout)


def kernel(**inputs):
    x = np.asarray(inputs["x"], np.float32)
    edge_index = np.asarray(inputs["edge_index"])
    eap = np.asarray(inputs["edge_attr_packed"])
    batch = np.asarray(inputs["batch"])
    W1a = np.ascontiguousarray(inputs["W1a"], np.float32)
    W1b = np.ascontiguousarray(inputs["W1b"], np.float32)
    W2a = np.ascontiguousarray(inputs["W2a"], np.float32)
    W2b = np.ascontiguousarray(inputs["W2b"], np.float32)
    b1a = np.asarray(inputs["b1a"], np.float32)
    b1b = np.asarray(inputs["b1b"], np.float32)
    b2a = np.asarray(inputs["b2a"], np.float32)
    b2b = np.asarray(inputs["b2b"], np.float32)
    root1 = np.asarray(inputs["root1"], np.float32)
    bias1 = np.asarray(inputs["bias1"], np.float32)
    root2 = np.asarray(inputs["root2"], np.float32)
    bias2 = np.asarray(inputs["bias2"], np.float32)

    # MSB-first bit unpack -> [E, 16]
    shifts = np.arange(7, -1, -1, dtype=np.int32)
    ea = ((eap[:, :, None].astype(np.int32) >> shifts) & 1).reshape(E, -1)
    ea = ea.astype(np.float32)

    src, dst = edge_index[0], edge_index[1]

    W1b_r = _reorder_oi(W1b, F_IN, H)
    msg1 = _run_layer("A", ea, x[src], W1a, b1a, W1b_r, b1b, H)
    h = _relu(_segmean(msg1, dst, N) + x @ root1 + bias1)

    W2b_r = _reorder_oi(W2b, H, H2)
    msg2 = _run_layer("B", ea, h[src], W2a, b2a, W2b_r, b2b, H2)
    h = _relu(_segmean(msg2, dst, N) + h @ root2 + bias2)

    g = _segmean(h, batch, NG)
    g = _relu(g @ np.asarray(inputs["fcW1"], np.float32) + np.asarray(inputs["fcb1"], np.float32))
    g = _relu(g @ np.asarray(inputs["fcW2"], np.float32) + np.asarray(inputs["fcb2"], np.float32))
    g = _relu(g @ np.asarray(inputs["fcW3"], np.float32) + np.asarray(inputs["fcb3"], np.float32))
    return (g @ np.asarray(inputs["fcW4"], np.float32) + np.asarray(inputs["fcb4"], np.float32)).astype(np.float32)


# revision 33
# speedup vs baseline: 10.2878x; 1.0449x over previous
"""GCN-with-edge-features kernel for 8 Trainium2 cores.

Data-parallel over edges (12500/core, padded to 12800). Two device
launches:
  A: h1 = relu(ea@W1a+b1a); theta1 = h1@W1b+b1b (kept in PSUM);
     msg1[e,o] = sum_i x[src[e],i] * theta1[e,i,o]   -> [E, 32]
  B: same with W2a/W2b and h[src]                    -> [E, 64]
Matmuls run as float32r (1 cycle/row). theta never leaves the chip;
the per-edge contraction is a DVE strided multiply + reduce with edges
on partitions. Host does only the segment-mean scatters, the tiny
root/bias adds, and the final 2000-row MLP.
"""
import numpy as np

import sys
for p in ("/opt/trn_rl_repo",):
    if p not in sys.path:
        sys.path.append(p)

from concourse import bass, bacc, mybir, tile
from concourse import bass_utils

E = 100000
N = 50000
NG = 2000
F_IN = 32
EF = 16
H = 32
H2 = 64
NC = 8
EPC = E // NC          # 12500 edges per core
CH = 512
NCHUNK = 25
EP = CH * NCHUNK       # 12800 padded edges per core
EB = CH // 128         # 4 edge blocks per chunk
D1 = H * F_IN          # 1024
D2 = H * H2            # 2048

_F32 = mybir.dt.float32
_F32R = mybir.dt.float32r
_F8 = mybir.dt.float8e4
_BF16 = mybir.dt.bfloat16
_DR = mybir.MatmulPerfMode.DoubleRow

USE_FP8 = True          # fp8e4m3 + DoubleRow for the big GEMMs
FP8_WSCALE = 64.0       # Wb pre-scale (W ~ +-1/32 underflows e4m3)
WA_SCALE = 16.0         # Wa pre-scale for the fp8 edge-net input GEMM
MUL_ENGINE = "vector"   # engine for the per-edge multiply
_RELU = mybir.ActivationFunctionType.Relu
_AX_X = mybir.AxisListType.X
_MUL = mybir.AluOpType.mult
_ADD = mybir.AluOpType.add

_NC_CACHE = {}


def _build_layer(tag, fin, fout):
    """One NNConv layer program: edge-net MLP + per-edge contraction.

    Orientation: theta^T [(o,i), e] with (o,i) on PSUM partitions.
      hm = relu(ea @ Wa + ba)              [1024, e]   (fp8 out, x WA_SCALE)
      thetaT = Wb'.T @ hm                  [(o,i), e]  per 128-row tile
      prod = thetaT * xsrep                (DVE, one pass, bf16 out)
      msgT[o,e] = sum_i prod[(o,i),e]      (PE selector matmul R_t)
    Wb' cols are (o,i)-ordered (col o*fin+i, scaled by FP8_WSCALE); R_t
    carries the 1/(scales) descale. Edge-net bias applied host-side.
    fp8 path: ea/Wa are DoubleRow-packed [8, 2, .] (row 2p+s on
    partition p sub s), Wa scaled by WA_SCALE.
    """
    DW = fin * fout                 # 1024 or 2048
    NT = DW // 128                  # 8 or 16 thetaT tiles
    NB = 5                          # chunks per batched xsrep/msgT DMA
    wdt = _F8 if USE_FP8 else _F32R
    descale = 1.0 / (FP8_WSCALE * WA_SCALE) if USE_FP8 else 1.0

    nc = bacc.Bacc(None, target_bir_lowering=False)

    if USE_FP8:
        eaT_d = nc.dram_tensor("eaT", [8, 2 * EP], _F8, kind="ExternalInput")
        Wa_d = nc.dram_tensor("Wa", [8, 2 * D1], _F8, kind="ExternalInput")
    else:
        eaT_d = nc.dram_tensor("eaT", [EF, EP], _F32R, kind="ExternalInput")
        Wa_d = nc.dram_tensor("Wa", [EF, D1], _F32R, kind="ExternalInput")
    xsT_d = nc.dram_tensor("xsT", [fin, EP], _BF16, kind="ExternalInput")
    ba_d = nc.dram_tensor("ba", [128, 8], _F32, kind="ExternalInput")
    Wb_d = nc.dram_tensor("Wb", [D1, DW], wdt, kind="ExternalInput")
    rdt = _F8 if USE_FP8 else _BF16
    R_d = nc.dram_tensor("R", [128, NT * fout], rdt, kind="ExternalInput")
    msgT_d = nc.dram_tensor("msgT", [fout, EP], _F32, kind="ExternalOutput")

    psh_b, pst_b, psm_b = (2, 2, 2) if tag == "A" else (1, 3, 1)
    with tile.TileContext(nc) as tc:
        with (
            tc.tile_pool(name="w", bufs=1) as wpool,
            tc.tile_pool(name="h", bufs=2) as hpool,
            tc.tile_pool(name="xs", bufs=2) as xspool,
            tc.tile_pool(name="pr", bufs=6) as prpool,
            tc.tile_pool(name="o", bufs=2) as opool,
            tc.tile_pool(name="psh", bufs=psh_b, space=bass.MemorySpace.PSUM) as pshpool,
            tc.tile_pool(name="pst", bufs=pst_b, space=bass.MemorySpace.PSUM) as pstpool,
            tc.tile_pool(name="psm", bufs=psm_b, space=bass.MemorySpace.PSUM) as psmpool,
        ):
            if USE_FP8:
                wa = wpool.tile([8, 2 * D1], _F8)
                ea_all = wpool.tile([8, 2 * EP], _F8)
                wa3 = wa[:].rearrange("p (s d) -> p s d", s=2)
                ea3_all = ea_all[:].rearrange("p (s e) -> p s e", s=2)
            else:
                wa = wpool.tile([EF, D1], _F32R)
                ea_all = wpool.tile([EF, EP], _F32R)
            nc.sync.dma_start(wa[:], Wa_d[:])
            nc.sync.dma_start(ea_all[:], eaT_d[:])
            ba = wpool.tile([128, 8], _F32)
            nc.sync.dma_start(ba[:], ba_d[:])
            # xsrep_all[p, e] = xs[p % fin, e] for the whole padded edge
            # range: 128//fin plain row-block copies (stride-0 broadcast
            # DMA misbehaves on hw). bf16 to halve the prologue DMA.
            xsrep_all = wpool.tile([128, EP], _BF16)
            for r in range(128 // fin):
                nc.sync.dma_start(xsrep_all[r * fin:(r + 1) * fin, :], xsT_d[:])
            wb = wpool.tile([128, 8 * DW], wdt)
            for k in range(8):
                nc.sync.dma_start(
                    wb[:, k * DW:(k + 1) * DW], Wb_d[k * 128:(k + 1) * 128, :]
                )
            R = wpool.tile([128, NT * fout], rdt)
            nc.sync.dma_start(R[:], R_d[:])

            wb3 = wb[:].rearrange("p (t d) -> p t d", t=8)

            def emit_layer_a(c):
                # hm = relu(Wa.T @ ea + ba): [1024, CH] as 8 tiles
                s = c * CH
                hm = hpool.tile([128, 8 * CH], wdt)
                for j in range(8):
                    ps = pshpool.tile([128, CH], _F32)
                    if USE_FP8:
                        nc.tensor.matmul(
                            ps[:],
                            wa3[:, :, j * 128:(j + 1) * 128],
                            ea3_all[:, :, s:s + CH],
                            start=True, stop=True,
                            perf_mode=_DR,
                        )
                    else:
                        nc.tensor.matmul(
                            ps[:],
                            wa[:, j * 128:(j + 1) * 128],
                            ea_all[:, s:s + CH],
                            start=True, stop=True,
                        )
                    nc.scalar.activation(
                        hm[:, j * CH:(j + 1) * CH], ps[:], _RELU,
                        bias=ba[:, j:j + 1],
                    )
                return hm

            hm_next = emit_layer_a(0)
            for c in range(NCHUNK):
                s = c * CH
                if c % NB == 0:
                    msgsb_b = opool.tile([fout, NB * CH], _F32)
                xsrep = xsrep_all[:, s:s + CH]

                hm = hm_next
                if c + 1 < NCHUNK:
                    hm_next = emit_layer_a(c + 1)
                hm3 = hm[:].rearrange("p (t e) -> p t e", t=8)

                msgps = psmpool.tile([fout, CH], _F32)
                NP = NT // 2   # theta-tile pairs

                def emit_pair(j):
                    # two theta tiles (2j, 2j+1) into one 2-bank PSUM tile,
                    # one fused scale+mul into fp8 prod for both.
                    P = pstpool.tile([128, 2 * CH], _F32)
                    for h in range(2):
                        t = 2 * j + h
                        if USE_FP8:
                            for g in range(4):
                                nc.tensor.matmul(
                                    P[:, h * CH:(h + 1) * CH],
                                    wb3[:, 2 * g:2 * g + 2, t * 128:(t + 1) * 128],
                                    hm3[:, 2 * g:2 * g + 2, :],
                                    start=(g == 0), stop=(g == 3),
                                    perf_mode=_DR,
                                )
                        else:
                            for k in range(8):
                                nc.tensor.matmul(
                                    P[:, h * CH:(h + 1) * CH],
                                    wb3[:, k, t * 128:(t + 1) * 128],
                                    hm3[:, k, :],
                                    start=(k == 0), stop=(k == 7),
                                )
                    prod = prpool.tile([128, 2 * CH], _F8 if USE_FP8 else _BF16)
                    nc.vector.scalar_tensor_tensor(
                        prod[:].rearrange("p (s e) -> p s e", s=2),
                        P[:].rearrange("p (s e) -> p s e", s=2),
                        descale,
                        xsrep.unsqueeze(1).broadcast_to([128, 2, CH]),
                        _MUL, _MUL,
                    )
                    return prod

                def emit_reduce(j, prod):
                    if USE_FP8:
                        # DoubleRow: sub s = theta tile 2j+s
                        nc.tensor.matmul(
                            msgps[:],
                            R[:, 2 * j * fout:(2 * j + 2) * fout]
                            .rearrange("p (s o) -> p s o", s=2),
                            prod[:].rearrange("p (s e) -> p s e", s=2),
                            start=(j == 0), stop=(j == NP - 1),
                            perf_mode=_DR,
                        )
                    else:
                        for h in range(2):
                            t = 2 * j + h
                            nc.tensor.matmul(
                                msgps[:], R[:, t * fout:(t + 1) * fout],
                                prod[:, h * CH:(h + 1) * CH],
                                start=(t == 0), stop=(t == NT - 1),
                            )

                # software-pipeline: keep mains ahead of each reduce so the
                # PE never head-of-line blocks on the DVE.
                prods = {}
                for j in range(NP):
                    prods[j] = emit_pair(j)
                    if j >= 1:
                        emit_reduce(j - 1, prods.pop(j - 1))
                emit_reduce(NP - 1, prods.pop(NP - 1))
                nc.scalar.copy(
                    msgsb_b[:, (c % NB) * CH:(c % NB + 1) * CH], msgps[:])
                if c % NB == NB - 1:
                    nc.sync.dma_start(
                        msgT_d[:, (c - NB + 1) * CH:(c + 1) * CH], msgsb_b[:])

    nc.compile()
    return nc


def _get_nc(tag):
    if tag not in _NC_CACHE:
        fin, fout = (F_IN, H) if tag == "A" else (H, H2)
        _NC_CACHE[tag] = _build_layer(tag, fin, fout)
    return _NC_CACHE[tag]


def compiled_ncs():
    return [_get_nc("A"), _get_nc("B")]


def _relu(v):
    return np.maximum(v, 0.0)


def _segmean(vals, idx, n):
    s = np.zeros((n, vals.shape[1]), np.float32)
    np.add.at(s, idx, vals)
    c = np.bincount(idx, minlength=n).astype(np.float32)
    return s / np.maximum(c, 1.0)[:, None]


def _reorder_oi(Wb, fin, fout):
    """Reorder edge-net output cols from (i, o) = i*fout + o to (o, i) =
    o*fin + i, so each 512-col block is 16 complete outputs."""
    W = Wb.reshape(D1, fin, fout).transpose(0, 2, 1).reshape(D1, fin * fout)
    return np.ascontiguousarray(W)


def _run_layer(tag, ea, feat_src, Wa, ba, Wb_r, bb, fout):
    """ea: [E,16] f32; feat_src: [E, fin] f32 (features gathered at src).
    bb: raw edge-net output bias [fin*fout] - applied host-side as
    msg += feat_src @ bb.reshape(fin, fout)."""
    import ml_dtypes
    fin = feat_src.shape[1]
    nc = _get_nc(tag)
    if USE_FP8:
        baT = np.ascontiguousarray((ba * WA_SCALE).reshape(8, 128).T)
        Wb_fin = (Wb_r * FP8_WSCALE).astype(ml_dtypes.float8_e4m3)
        Wa_fin = np.ascontiguousarray(
            (Wa * WA_SCALE).reshape(8, 2, D1)).astype(ml_dtypes.float8_e4m3)
    else:
        baT = np.ascontiguousarray(ba.reshape(8, 128).T)
        Wb_fin = Wb_r
        Wa_fin = Wa
    # R_t[p, o] = (o == t*(128//fin) + p//fin), stacked over t (the fp8
    # descale is applied inside the device stt op)
    NT = (fin * fout) // 128
    R = np.zeros((128, NT * fout), np.float32)
    for t in range(NT):
        for p in range(128):
            R[p, t * fout + t * (128 // fin) + p // fin] = 1.0
    R = R.astype(ml_dtypes.float8_e4m3 if USE_FP8 else ml_dtypes.bfloat16)
    in_maps = []
    for i in range(NC):
        eaT = np.zeros((EF, EP), np.float32)
        eaT[:, :EPC] = ea[i * EPC:(i + 1) * EPC].T
        if USE_FP8:
            eaT = np.ascontiguousarray(
                eaT.reshape(8, 2, EP)).astype(ml_dtypes.float8_e4m3)
        xsT = np.zeros((fin, EP), np.float32)
        xsT[:, :EPC] = feat_src[i * EPC:(i + 1) * EPC].T
        xsT = xsT.astype(ml_dtypes.bfloat16)
        in_maps.append(dict(eaT=eaT, xsT=xsT, Wa=Wa_fin, ba=baT, Wb=Wb_fin, R=R))
    res = bass_utils.run_bass_kernel_spmd(nc, in_maps, core_ids=list(range(NC)))
    msg = np.concatenate(
        [res.results[i]["msgT"][:, :EPC].T for i in range(NC)], axis=0)
    return msg + feat_src @ bb.reshape(fin, fout)


def kernel(**inputs):
    x = np.asarray(inputs["x"], np.float32)
    edge_index = np.asarray(inputs["edge_index"])
    eap = np.asarray(inputs["edge_attr_packed"])
    batch = np.asarray(inputs["batch"])
    W1a = np.ascontiguousarray(inputs["W1a"], np.float32)
    W1b = np.ascontiguousarray(inputs["W1b"], np.float32)
    W2a = np.ascontiguousarray(inputs["W2a"], np.float32)
    W2b = np.ascontiguousarray(inputs["W2b"], np.float32)
    b1a = np.asarray(inputs["b1a"], np.float32)
    b1b = np.asarray(inputs["b1b"], np.float32)
    b2a = np.asarray(inputs["b2a"], np.float32)
    b2b = np.asarray(inputs["b2b"], np.float32)
    root1 = np.asarray(inputs["root1"], np.float32)
    bias1 = np.asarray(inputs["bias1"], np.float32)
    root2 = np.asarray(inputs["root2"], np.float32)
    bias2 = np.asarray(inputs["bias2"], np.float32)

    # MSB-first bit unpack -> [E, 16]
    shifts = np.arange(7, -1, -1, dtype=np.int32)
    ea = ((eap[:, :, None].astype(np.int32) >> shifts) & 1).reshape(E, -1)
    ea = ea.astype(np.float32)

    src, dst = edge_index[0], edge_index[1]

    W1b_r = _reorder_oi(W1b, F_IN, H)
    msg1 = _run_layer("A", ea, x[src], W1a, b1a, W1b_r, b1b, H)
    h = _relu(_segmean(msg1, dst, N) + x @ root1 + bias1)

    W2b_r = _reorder_oi(W2b, H, H2)
    msg2 = _run_layer("B", ea, h[src], W2a, b2a, W2b_r, b2b, H2)
    h = _relu(_segmean(msg2, dst, N) + h @ root2 + bias2)

    g = _segmean(h, batch, NG)
    g = _relu(g @ np.asarray(inputs["fcW1"], np.float32) + np.asarray(inputs["fcb1"], np.float32))
    g = _relu(g @ np.asarray(inputs["fcW2"], np.float32) + np.asarray(inputs["fcb2"], np.float32))
    g = _relu(g @ np.asarray(inputs["fcW3"], np.float32) + np.asarray(inputs["fcb3"], np.float32))
    return (g @ np.asarray(inputs["fcW4"], np.float32) + np.asarray(inputs["fcb4"], np.float32)).astype(np.float32)
